# revision 16
# baseline (speedup 1.0000x reference)
"""Trainium2 Bass kernel for DistributedAFNO2D (v2).

out = irfft2( softshrink( W2*relu(W1*rfft2(x) + b1) + b2 ) ) + x
Block k -> core k; per core (2, 96, 256, 256), dense DFT matmuls.

v2 changes vs baseline:
- Phase A exploits Hermitian symmetry of the h-DFT of real x: S1 computes only
  u=0..128 (free 258 vs 512); the u=129..255 half of S2 is obtained from the
  mirrored/conjugated Y columns via R1/-R2 (rows stored un-reversed; phase C
  uses mirrored inverse-DFT constants for that chunk).
- Phase B processes 4 u-rows at a time with r/i-split [96, 512] tiles
  (4 big matmuls per layer instead of 12 small), relu/clamp/sub spread across
  scalar/vector/gpsimd engines. v=128 (Nyquist) and the v=0 DC inverse column
  are handled in batched side-paths using PE transposes (no DRAM scatter).
- DC term of the inverse (v=0) computed for all channels at once (8 matmuls
  per batch instead of 8 per channel).
- bf16 residual + bf16 output (halves residual/store DMA traffic).
- DMA spread across sync(loads)/gpsimd+scalar(stores) queues, merged patterns.
"""
import sys
import numpy as np

sys.path.insert(0, "/opt/trn_rl_repo")

import ml_dtypes

BF16 = ml_dtypes.bfloat16
FP8 = ml_dtypes.float8_e4m3fn

H = 256
W = 256
NV = 129
BLK = 96
NCORES = 8
B = 2
LAM = 0.01
OUT_KEY = "outbf"


def make_host_consts():
    I = np.eye(H)
    F = np.fft.fft(I, axis=0, norm='ortho')       # F[u,h]
    Fi = np.fft.ifft(I, axis=0, norm='ortho')     # Fi[h,u]
    CHr = F.real.T.copy()                          # [h,u]
    CHi = F.imag.T.copy()
    EWr = F.real.T[:, :NV].copy()                  # [w,v]
    EWi = F.imag.T[:, :NV].copy()
    CHIr = Fi.real.T.copy()                        # [u,h]
    CHIi = Fi.imag.T.copy()
    Ir = np.eye(NV)
    Gc = np.fft.irfft(Ir, n=W, axis=-1, norm='ortho')        # [v,w]
    Gs = np.fft.irfft(1j * Ir, n=W, axis=-1, norm='ortho')   # [v,w]

    c = {}
    c['chS'] = np.stack([
        np.concatenate([CHr[j*128:(j+1)*128, :NV], CHi[j*128:(j+1)*128, :NV]], axis=1)
        for j in range(2)])                        # [hc][128, 258]
    c['r1'] = np.stack([
        np.concatenate([EWr[j*128:(j+1)*128], EWi[j*128:(j+1)*128]], axis=1)
        for j in range(2)])                        # [wc][128, 258]
    r2 = np.stack([
        np.concatenate([-EWi[j*128:(j+1)*128], EWr[j*128:(j+1)*128]], axis=1)
        for j in range(2)])
    c['r2'] = r2
    c['r2n'] = -r2
    m0 = np.arange(128)
    m1 = 255 - np.arange(128)                      # mirror rows for u-chunk 1
    c['chip'] = np.stack([
        np.concatenate([CHIr[m0], CHIi[m0]], axis=1),
        np.concatenate([CHIr[m1], CHIi[m1]], axis=1)])   # [uc][128, 512]
    c['nchi'] = np.stack([-CHIi[m0], -CHIi[m1]])   # [uc][128, 256]
    c['gc'] = Gc[1:129]                            # [128, 256]
    c['gs'] = Gs[1:129]
    c['ident'] = np.eye(128)
    out = {k: v.astype(BF16) for k, v in c.items()}

    def interleave_pad(a0, a1, pitch):
        # [128, n] x2 -> [128, 2, pitch] -> [128, 2*pitch]
        n = a0.shape[1]
        r = np.zeros((128, 2, pitch), dtype=np.float64)
        r[:, 0, :n] = a0
        r[:, 1, :n] = a1
        return r.reshape(128, 2*pitch)

    f8 = {}
    f8['chSdr'] = interleave_pad(c['chS'][0], c['chS'][1], 272)
    f8['r1dr'] = interleave_pad(c['r1'][0], c['r1'][1], 272)
    f8['r2dr'] = interleave_pad(c['r2'][0], c['r2'][1], 272)
    f8['r2ndr'] = interleave_pad(c['r2n'][0], c['r2n'][1], 272)
    f8['chipRdr'] = interleave_pad(c['chip'][0][:, 0:256], c['chip'][1][:, 0:256], 256)
    f8['chipIdr'] = interleave_pad(c['chip'][0][:, 256:512], c['chip'][1][:, 256:512], 256)
    f8['nchidr'] = interleave_pad(c['nchi'][0], c['nchi'][1], 256)
    for k, v in f8.items():
        out[k] = v.astype(FP8)
    return out


def build_nc():
    import concourse.bass as bass
    import concourse.tile as tile
    from concourse import bacc, mybir

    dt = mybir.dt
    nc = bacc.Bacc("TRN2", target_bir_lowering=False, debug=False)

    xbf = nc.dram_tensor("xbf", [B, BLK, H, W], dt.bfloat16, kind="ExternalInput").ap()
    chS = nc.dram_tensor("chS", [2, 128, 258], dt.bfloat16, kind="ExternalInput").ap()
    r1 = nc.dram_tensor("r1", [2, 128, 258], dt.bfloat16, kind="ExternalInput").ap()
    r2 = nc.dram_tensor("r2", [2, 128, 258], dt.bfloat16, kind="ExternalInput").ap()
    r2n = nc.dram_tensor("r2n", [2, 128, 258], dt.bfloat16, kind="ExternalInput").ap()
    chip = nc.dram_tensor("chip", [2, 128, 512], dt.bfloat16, kind="ExternalInput").ap()
    nchi = nc.dram_tensor("nchi", [2, 128, 256], dt.bfloat16, kind="ExternalInput").ap()
    gc = nc.dram_tensor("gc", [128, 256], dt.bfloat16, kind="ExternalInput").ap()
    gs = nc.dram_tensor("gs", [128, 256], dt.bfloat16, kind="ExternalInput").ap()
    ident = nc.dram_tensor("ident", [128, 128], dt.bfloat16, kind="ExternalInput").ap()
    wts = {n: nc.dram_tensor(n, [96, 96], dt.bfloat16, kind="ExternalInput").ap()
           for n in ['w1r', 'w1i', 'w1in', 'w2r', 'w2i', 'w2in']}
    b1cols = nc.dram_tensor("b1cols", [96, 2], dt.float32, kind="ExternalInput").ap()
    b2cols = nc.dram_tensor("b2cols", [96, 4], dt.float32, kind="ExternalInput").ap()
    outbf = nc.dram_tensor("outbf", [B, BLK, H, W], dt.bfloat16, kind="ExternalOutput").ap()

    # DRAM scratch
    zbuf = [nc.dram_tensor(f"zbuf{b}", [H, BLK, 256], dt.bfloat16).ap() for b in range(B)]
    sbufd = [nc.dram_tensor(f"sbufd{b}", [BLK, H, 258], dt.bfloat16).ap() for b in range(B)]

    RELU = None  # set below

    with tile.TileContext(nc) as tc:
        from contextlib import ExitStack
        with ExitStack() as ctx:
            consts = ctx.enter_context(tc.tile_pool(name="consts", bufs=1))
            pa = ctx.enter_context(tc.tile_pool(name="pa", bufs=4))
            pb = ctx.enter_context(tc.tile_pool(name="pb", bufs=4))
            pc = ctx.enter_context(tc.tile_pool(name="pc", bufs=6))
            pst = ctx.enter_context(tc.tile_pool(name="pst", bufs=2))  # persistent per-batch
            psum = ctx.enter_context(tc.tile_pool(name="psum", bufs=1, space="PSUM"))

            RELU = mybir.ActivationFunctionType.Relu
            ADD = mybir.AluOpType.add
            MAXOP = mybir.AluOpType.max
            MINOP = mybir.AluOpType.min
            SUB = mybir.AluOpType.subtract

            def split_hc(ap2d, n=2):
                # [ (n h) w ] -> [h, n, w] permuted AP
                return ap2d.rearrange("(n h) w -> n h w", n=n).transpose([1, 0, 2])

            _cload_rr = [0]
            _cload_engs = None  # set after engines known

            def cload(name, ap_, shape):
                t = consts.tile(list(shape), dt.bfloat16, tag=name, name=name)
                eng = (nc.sync, nc.gpsimd, nc.scalar)[_cload_rr[0] % 3]
                _cload_rr[0] += 1
                eng.dma_start(out=t, in_=ap_)
                return t

            t_chS = [cload(f"chS{j}", chS[j], [128, 258]) for j in range(2)]
            t_r1 = [cload(f"r1{j}", r1[j], [128, 258]) for j in range(2)]
            t_r2 = [cload(f"r2{j}", r2[j], [128, 258]) for j in range(2)]
            t_r2n = [cload(f"r2n{j}", r2n[j], [128, 258]) for j in range(2)]
            t_chip = [cload(f"chip{j}", chip[j], [128, 512]) for j in range(2)]
            t_nchi = [cload(f"nchi{j}", nchi[j], [128, 256]) for j in range(2)]
            t_gc = cload("gc", gc, [128, 256])
            t_gs = cload("gs", gs, [128, 256])
            t_id = cload("ident", ident, [128, 128])
            t_w = {n: cload(n, ap_, [96, 96]) for n, ap_ in wts.items()}
            t_b1 = consts.tile([96, 2], dt.float32, tag="b1", name="t_b1")
            nc.sync.dma_start(out=t_b1, in_=b1cols)
            t_b2 = consts.tile([96, 4], dt.float32, tag="b2", name="t_b2")
            nc.sync.dma_start(out=t_b2, in_=b2cols)

            persist = {}

            def get_persist(b):
                if b in persist:
                    return persist[b]
                p = dict(
                    znyqA=[pst.tile([128, 192], dt.bfloat16, tag=f"znyqA{uc}", name=f"znyqA{uc}")
                           for uc in range(2)],
                    zN=pst.tile([96, 512], dt.bfloat16, tag="zN", name="zN"),
                    s0acc=pst.tile([96, 512], dt.bfloat16, tag="s0acc", name="s0acc"),
                    snyqT=[pst.tile([128, 192], dt.bfloat16, tag=f"snyqT{uc}", name=f"snyqT{uc}")
                           for uc in range(2)],
                    q0sb=[pst.tile([128, 96], dt.float32, tag=f"q0sb{hc}", name=f"q0sb{hc}")
                          for hc in range(2)],
                )
                persist[b] = p
                return p

            def emit_A_channel(b, c, ztag="pgA"):
                P = get_persist(b)
                znyqA = P['znyqA']
                xt = pa.tile([128, 512], dt.bfloat16, tag="xt", name="xt")
                nc.sync.dma_start(out=xt, in_=split_hc(xbf[b, c]))
                ys = []
                for wc in range(2):
                    psy = psum.tile([128, 258], dt.float32, tag="pgA", name="psy", bufs=4)
                    nc.tensor.matmul(psy, lhsT=xt[:, wc*128:wc*128+128],
                                     rhs=t_chS[0], start=True, stop=False)
                    nc.tensor.matmul(psy, lhsT=xt[:, 256+wc*128:256+wc*128+128],
                                     rhs=t_chS[1], start=False, stop=True)
                    y = pa.tile([128, 258], dt.bfloat16, tag=f"y{wc}", name=f"y{wc}")
                    nc.scalar.copy(y, psy)
                    ys.append(y)

                ztm = pa.tile([128, 512], dt.bfloat16, tag="ztm", name="ztm")
                for uc in range(2):
                    psz = psum.tile([128, 258], dt.float32, tag=ztag, name="psz", bufs=4)
                    if uc == 0:
                        sl_r, sl_i, rB = slice(0, 128), slice(129, 257), t_r2
                    else:
                        sl_r, sl_i, rB = slice(1, 129), slice(130, 258), t_r2n
                    nc.tensor.matmul(psz, lhsT=ys[0][:, sl_r], rhs=t_r1[0], start=True, stop=False)
                    nc.tensor.matmul(psz, lhsT=ys[0][:, sl_i], rhs=rB[0], start=False, stop=False)
                    nc.tensor.matmul(psz, lhsT=ys[1][:, sl_r], rhs=t_r1[1], start=False, stop=False)
                    nc.tensor.matmul(psz, lhsT=ys[1][:, sl_i], rhs=rB[1], start=False, stop=True)
                    pszv = psz.rearrange("p (ri v) -> p ri v", ri=2)
                    nc.vector.tensor_scalar_add(
                        ztm[:, uc*256:(uc+1)*256].rearrange("p (ri v) -> p ri v", ri=2),
                        pszv[:, :, 0:128], 0.0)
                    nc.scalar.copy(
                        znyqA[uc].rearrange("p (ri c) -> p ri c", ri=2)[:, :, c],
                        pszv[:, :, 128])
                nc.gpsimd.dma_start(out=split_hc(zbuf[b][:, c, :]), in_=ztm)

            def emit_A_end(b):
                P = get_persist(b)
                for uc in range(2):
                    for ri in range(2):
                        tp = psum.tile([96, 128], dt.bfloat16, tag="pgB", name="tpz", bufs=4)
                        nc.tensor.transpose(tp, P['znyqA'][uc][:, ri*96:(ri+1)*96], t_id)
                        nc.scalar.copy(P['zN'][:, ri*256+uc*128:ri*256+uc*128+128], tp)

            b1r, b1i = t_b1[:, 0:1], t_b1[:, 1:2]

            def emit_B_group(b, g):
                P = get_persist(b)
                r0 = 4*g
                zri = pb.tile([96, 1024], dt.bfloat16, tag="zri", name="zri")
                zsrc = zbuf[b][r0:r0+4, :, :].rearrange("u c (ri v) -> u c ri v", ri=2)
                for ri in range(2):
                    nc.sync.dma_start(
                        out=zri[:, ri*512:(ri+1)*512],
                        in_=zsrc[:, :, ri, :].transpose([1, 0, 2]))
                zr, zi = zri[:, 0:512], zri[:, 512:1024]

                ps1r = psum.tile([96, 512], dt.float32, tag="pgB", name="ps1r", bufs=4)
                ps1i = psum.tile([96, 512], dt.float32, tag="pgB", name="ps1i", bufs=4)
                nc.tensor.matmul(ps1r, lhsT=t_w['w1r'], rhs=zr, start=True, stop=False)
                nc.tensor.matmul(ps1r, lhsT=t_w['w1in'], rhs=zi, start=False, stop=True)
                nc.tensor.matmul(ps1i, lhsT=t_w['w1i'], rhs=zr, start=True, stop=False)
                nc.tensor.matmul(ps1i, lhsT=t_w['w1r'], rhs=zi, start=False, stop=True)

                o1 = pb.tile([96, 1024], dt.bfloat16, tag="o1", name="o1")
                nc.scalar.activation(o1[:, 0:512], ps1r, RELU, bias=b1r)
                nc.vector.tensor_scalar(o1[:, 512:1024], ps1i, b1i, 0.0, ADD, MAXOP)

                ps2r = psum.tile([96, 512], dt.float32, tag="pgB", name="ps2r", bufs=4)
                ps2i = psum.tile([96, 512], dt.float32, tag="pgB", name="ps2i", bufs=4)
                nc.tensor.matmul(ps2r, lhsT=t_w['w2r'], rhs=o1[:, 0:512], start=True, stop=False)
                nc.tensor.matmul(ps2r, lhsT=t_w['w2in'], rhs=o1[:, 512:1024], start=False, stop=True)
                nc.tensor.matmul(ps2i, lhsT=t_w['w2i'], rhs=o1[:, 0:512], start=True, stop=False)
                nc.tensor.matmul(ps2i, lhsT=t_w['w2r'], rhs=o1[:, 512:1024], start=False, stop=True)

                tt = pb.tile([96, 1024], dt.bfloat16, tag="tt", name="tt")
                nc.scalar.copy(tt[:, 0:512], ps2r)
                nc.scalar.copy(tt[:, 512:1024], ps2i)
                clr = pb.tile([96, 512], dt.bfloat16, tag="clr", name="clr")
                cli = pb.tile([96, 512], dt.bfloat16, tag="cli", name="cli")
                nc.gpsimd.tensor_scalar(clr, tt[:, 0:512], t_b2[:, 0:1], t_b2[:, 1:2], MINOP, MAXOP)
                nc.gpsimd.tensor_scalar(cli, tt[:, 512:1024], t_b2[:, 2:3], t_b2[:, 3:4], MINOP, MAXOP)
                stt = pb.tile([96, 1024], dt.bfloat16, tag="stt", name="stt")
                nc.vector.tensor_tensor(stt[:, 0:512], tt[:, 0:512], clr, SUB)
                nc.vector.tensor_tensor(stt[:, 512:1024], tt[:, 512:1024], cli, SUB)
                sttv = stt.rearrange("p (ri u v) -> p ri u v", ri=2, u=4)
                nc.scalar.copy(
                    P['s0acc'].rearrange("p (ri r) -> p ri r", ri=2)[:, :, r0:r0+4],
                    sttv[:, :, :, 0])
                sdst = sbufd[b][:, r0:r0+4, :].rearrange("c u (ri v) -> c u ri v", ri=2)
                for ri, deng in ((0, nc.gpsimd), (1, nc.sync)):
                    deng.dma_start(
                        out=sdst[:, :, ri, 0:128],
                        in_=stt[:, ri*512:(ri+1)*512])

            def emit_B_tail(b):
                P = get_persist(b)
                zN, s0acc, snyqT, q0sb = P['zN'], P['s0acc'], P['snyqT'], P['q0sb']
                zNr, zNi = zN[:, 0:256], zN[:, 256:512]
                psnr = psum.tile([96, 256], dt.float32, tag="pgB", name="psnr", bufs=4)
                psni = psum.tile([96, 256], dt.float32, tag="pgB", name="psni", bufs=4)
                nc.tensor.matmul(psnr, lhsT=t_w['w1r'], rhs=zNr, start=True, stop=False)
                nc.tensor.matmul(psnr, lhsT=t_w['w1in'], rhs=zNi, start=False, stop=True)
                nc.tensor.matmul(psni, lhsT=t_w['w1i'], rhs=zNr, start=True, stop=False)
                nc.tensor.matmul(psni, lhsT=t_w['w1r'], rhs=zNi, start=False, stop=True)
                o1n = pb.tile([96, 512], dt.bfloat16, tag="o1n", name="o1n")
                nc.scalar.activation(o1n[:, 0:256], psnr, RELU, bias=b1r)
                nc.vector.tensor_scalar(o1n[:, 256:512], psni, b1i, 0.0, ADD, MAXOP)
                psnr2 = psum.tile([96, 256], dt.float32, tag="pgB", name="psnr2", bufs=4)
                psni2 = psum.tile([96, 256], dt.float32, tag="pgB", name="psni2", bufs=4)
                nc.tensor.matmul(psnr2, lhsT=t_w['w2r'], rhs=o1n[:, 0:256], start=True, stop=False)
                nc.tensor.matmul(psnr2, lhsT=t_w['w2in'], rhs=o1n[:, 256:512], start=False, stop=True)
                nc.tensor.matmul(psni2, lhsT=t_w['w2i'], rhs=o1n[:, 0:256], start=True, stop=False)
                nc.tensor.matmul(psni2, lhsT=t_w['w2r'], rhs=o1n[:, 256:512], start=False, stop=True)
                ttn = pb.tile([96, 512], dt.bfloat16, tag="ttn", name="ttn")
                nc.scalar.copy(ttn[:, 0:256], psnr2)
                nc.scalar.copy(ttn[:, 256:512], psni2)
                clnr = pb.tile([96, 256], dt.bfloat16, tag="clnr", name="clnr")
                clni = pb.tile([96, 256], dt.bfloat16, tag="clni", name="clni")
                nc.gpsimd.tensor_scalar(clnr, ttn[:, 0:256], t_b2[:, 0:1], t_b2[:, 1:2], MINOP, MAXOP)
                nc.gpsimd.tensor_scalar(clni, ttn[:, 256:512], t_b2[:, 2:3], t_b2[:, 3:4], MINOP, MAXOP)
                sN = pb.tile([96, 512], dt.bfloat16, tag="sN", name="sN")
                nc.vector.tensor_tensor(sN[:, 0:256], ttn[:, 0:256], clnr, SUB)
                nc.vector.tensor_tensor(sN[:, 256:512], ttn[:, 256:512], clni, SUB)
                for uc in range(2):
                    for ri in range(2):
                        tp = psum.tile([128, 96], dt.bfloat16, tag="pgB", name="tps", bufs=4)
                        nc.tensor.transpose(tp, sN[:, ri*256+uc*128:ri*256+uc*128+128],
                                            t_id[0:96, 0:96])
                        nc.scalar.copy(snyqT[uc][:, ri*96:(ri+1)*96], tp)

                s0T = [pst.tile([128, 192], dt.bfloat16, tag=f"s0T{uc}", name=f"s0T{uc}")
                       for uc in range(2)]
                for uc in range(2):
                    for ri in range(2):
                        tp = psum.tile([128, 96], dt.bfloat16, tag="pgB", name="tp0", bufs=4)
                        nc.tensor.transpose(tp, s0acc[:, ri*256+uc*128:ri*256+uc*128+128],
                                            t_id[0:96, 0:96])
                        nc.scalar.copy(s0T[uc][:, ri*96:(ri+1)*96], tp)
                for hc in range(2):
                    psq = psum.tile([128, 96], dt.float32, tag="pgB", name="psq", bufs=4)
                    for uc in range(2):
                        nc.tensor.matmul(psq, lhsT=t_chip[uc][:, hc*128:(hc+1)*128],
                                         rhs=s0T[uc][:, 0:96], start=(uc == 0), stop=False)
                        nc.tensor.matmul(psq, lhsT=t_nchi[uc][:, hc*128:(hc+1)*128],
                                         rhs=s0T[uc][:, 96:192], start=False, stop=(uc == 1))
                    nc.vector.tensor_scalar_mul(q0sb[hc], psq, 1.0/16.0)

            def emit_C_channel(b, c):
                P = get_persist(b)
                snyqT, q0sb = P['snyqT'], P['q0sb']
                st = pc.tile([128, 516], dt.bfloat16, tag="st", name="st")
                nc.sync.dma_start(out=st, in_=split_hc(sbufd[b][c]))
                stv = st.rearrange("p (uc ri v) -> p uc ri v", uc=2, ri=2)
                for uc in range(2):
                    nc.gpsimd.tensor_scalar_add(
                        stv[:, uc, :, 128],
                        snyqT[uc].rearrange("p (ri c) -> p ri c", ri=2)[:, :, c], 0.0)

                psa = psum.tile([128, 256], dt.float32, tag="pgA", name="psa", bufs=4)
                psb = psum.tile([128, 256], dt.float32, tag="pgA", name="psb", bufs=4)
                nc.tensor.matmul(psa, lhsT=st[:, 1:129], rhs=t_chip[0][:, 0:256], start=True, stop=False)
                nc.tensor.matmul(psa, lhsT=st[:, 130:258], rhs=t_nchi[0], start=False, stop=False)
                nc.tensor.matmul(psa, lhsT=st[:, 259:387], rhs=t_chip[1][:, 0:256], start=False, stop=False)
                nc.tensor.matmul(psa, lhsT=st[:, 388:516], rhs=t_nchi[1], start=False, stop=True)
                nc.tensor.matmul(psb, lhsT=st[:, 1:129], rhs=t_chip[0][:, 256:512], start=True, stop=False)
                nc.tensor.matmul(psb, lhsT=st[:, 130:258], rhs=t_chip[0][:, 0:256], start=False, stop=False)
                nc.tensor.matmul(psb, lhsT=st[:, 259:387], rhs=t_chip[1][:, 256:512], start=False, stop=False)
                nc.tensor.matmul(psb, lhsT=st[:, 388:516], rhs=t_chip[1][:, 0:256], start=False, stop=True)

                qr = pc.tile([128, 256], dt.bfloat16, tag="qr", name="qr")
                nc.scalar.copy(qr, psa)
                qi = pc.tile([128, 256], dt.bfloat16, tag="qi", name="qi")
                nc.vector.tensor_scalar_add(qi, psb, 0.0)

                xres = pc.tile([128, 512], dt.bfloat16, tag="xres", name="xres")
                nc.sync.dma_start(out=xres, in_=split_hc(xbf[b, c]))
                ot = pc.tile([128, 512], dt.bfloat16, tag="ot", name="ot")
                for hc in range(2):
                    pso = psum.tile([128, 256], dt.float32, tag="pgB", name="pso", bufs=4)
                    nc.tensor.matmul(pso, lhsT=qr[:, hc*128:(hc+1)*128], rhs=t_gc, start=True, stop=False)
                    nc.tensor.matmul(pso, lhsT=qi[:, hc*128:(hc+1)*128], rhs=t_gs, start=False, stop=True)
                    nc.vector.scalar_tensor_tensor(
                        ot[:, hc*256:(hc+1)*256], xres[:, hc*256:(hc+1)*256],
                        q0sb[hc][:, c:c+1], pso, ADD, ADD)
                nc.scalar.dma_start(out=split_hc(outbf[b, c]), in_=ot)

            # ---- schedule: A0 | A1+B0 | C0+B1 | C1 ----
            for c in range(BLK):
                emit_A_channel(0, c, ztag="pgB")
            emit_A_end(0)
            gi = 0
            for i in range(BLK):
                emit_A_channel(1, i)
                while gi < (i+1)*64//BLK:
                    emit_B_group(0, gi)
                    gi += 1
            emit_A_end(1)
            emit_B_tail(0)
            gi = 0
            for i in range(BLK):
                emit_C_channel(0, i)
                while gi < (i+1)*64//BLK:
                    emit_B_group(1, gi)
                    gi += 1
            emit_B_tail(1)
            for c in range(BLK):
                emit_C_channel(1, c)
    nc.compile()
    return nc


_NC_CACHE = {}


def _get_nc():
    if 'nc' not in _NC_CACHE:
        _NC_CACHE['nc'] = build_nc()
    return _NC_CACHE['nc']


def make_in_maps(x, w1, b1, w2, b2):
    hc = make_host_consts()
    x = np.ascontiguousarray(x, dtype=np.float32)
    in_maps = []
    for k in range(NCORES):
        xk = np.ascontiguousarray(x[:, BLK*k:BLK*(k+1)])
        w1k, w2k = w1[k], w2[k]
        b1k = b1[k, :, 0, 0, :]
        b2k = b2[k, :, 0, 0, :]
        b2colsk = np.stack([LAM - b2k[:, 0], -LAM - b2k[:, 0],
                            LAM - b2k[:, 1], -LAM - b2k[:, 1]], axis=1).astype(np.float32)
        m = dict(
            xbf=xk.astype(BF16),
            chS=hc['chS'], r1=hc['r1'], r2=hc['r2'], r2n=hc['r2n'],
            chip=hc['chip'], nchi=hc['nchi'], gc=hc['gc'], gs=hc['gs'], ident=hc['ident'],
            w1r=w1k[..., 0].astype(BF16), w1i=w1k[..., 1].astype(BF16),
            w1in=(-w1k[..., 1]).astype(BF16),
            w2r=w2k[..., 0].astype(BF16), w2i=w2k[..., 1].astype(BF16),
            w2in=(-w2k[..., 1]).astype(BF16),
            b1cols=np.ascontiguousarray(b1k, dtype=np.float32),
            b2cols=b2colsk,
        )
        in_maps.append(m)
    return in_maps


def kernel(x, w1, b1, w2, b2):
    from concourse.bass_utils import run_bass_kernel_spmd
    nc = _get_nc()
    in_maps = make_in_maps(np.asarray(x), np.asarray(w1), np.asarray(b1),
                           np.asarray(w2), np.asarray(b2))
    res = run_bass_kernel_spmd(nc, in_maps, core_ids=list(range(NCORES)))
    outs = [np.asarray(res.results[k][OUT_KEY]).astype(np.float32) for k in range(NCORES)]
    return np.concatenate(outs, axis=1)
